# revision 13
# baseline (speedup 1.0000x reference)
"""Multi-headed self-attention (B=2, S=2048, D=1024, H=16) on 8 TRN2 cores.

Sharding: hybrid batch x head tensor-parallel. Core c handles batch c//4 and
heads (c%4)*4 .. (c%4)*4+3. Each core computes x = query[b] + pos_emb, the QKV
projection for its 4 heads, attention, and a partial output projection
(o_heads @ w_out_rows). Host sums the 4 partials per batch.

v5 design notes (fp32r baseline ~410us, v2 ~304, v3 ~279, v4 ~260):
- The kernel is ScalarE-bound: 4 heads x 2048^2 exps = 16.8M elements at
  1 elem/lane/cycle @ 1.2 GHz is a ~142us floor (N=1024 chunks). Everything
  else hides under the exp stream, which starts as early as the input DMAs
  allow (~4.5MB of critical bytes at ~390 GB/s) and runs wall-to-wall.
- Attention is a flat pipeline over 8 slots (2 query-halves x 4 heads); AV
  of slot i runs under QK+exp of slot i+1; exp'ed scores (ptiles) buffer in
  SBUF bf16 to decouple the chains. Only the projections the first exps
  need (Q half-0 and the first K columns) precede the stream; every other
  projection, the V transposes, and the ENTIRE query-half-0 output
  projection are deadline-scheduled PE filler inside the stream, then
  dummy matmuls keep the PE dense so the HAM clock gate stays at K=8/8
  (the fp32r baseline sat throttled at 1.2 GHz for its whole attention
  phase; identity-matmul dummies also pre-warm during the DMA head).
- PSUM: score tiles 2x[128,1024] (4 banks) + AV accumulator [65,1024]
  (2 banks) + two filler slots (2 banks) = 8.
- Softmax denominators ride the AV matmul as a ones-column (row DK of the
  accumulator). Off-critical-path heads: reciprocal on a [64,16] reshape of
  the denominator row + DMA partition-broadcast. The last head (critical
  path into the remaining output projection) broadcasts via a rank-1 PE
  matmul + reciprocal_approx_fast instead - no DMA round trip.
- Softmax skips the max-subtraction: scores * dk^-0.5 are bounded (~|12|)
  for these inputs, well within exp range.
"""

import os
import sys

import numpy as np

if "/opt/trn_rl_repo" not in sys.path:
    sys.path.insert(0, "/opt/trn_rl_repo")

B, S, D, H = 2, 2048, 1024, 16
DK = 64
P = 128
NCORES = 8
HPC = H // (NCORES // B)  # heads per core = 4
T = S  # tokens per core (one batch)
E = HPC * 3 * DK  # 768 qkv output columns per core
NDC = D // P  # 8 contraction chunks
NEC = E // P  # 6 projection output chunks
NTB = T // P  # 16 token blocks
NTG = T // 512  # 4 token groups of 512
QH = T // 2  # query half
SCALE = DK**-0.5

_CACHE = {}


def _build_program(reps=1):
    from contextlib import ExitStack, nullcontext

    import concourse.bass as bass
    import concourse.tile as tile
    from concourse import bacc
    from concourse import mybir
    from concourse.masks import make_identity

    f32 = mybir.dt.float32
    bf16 = mybir.dt.bfloat16
    EXP = mybir.ActivationFunctionType.Exp

    nc = bacc.Bacc()
    xqT = nc.declare_dram_parameter("xqT", [D, T], bf16, isOutput=False)
    posT = nc.declare_dram_parameter("posT", [D, T], bf16, isOutput=False)
    wqkv = nc.declare_dram_parameter("wqkv", [D, E], bf16, isOutput=False)
    wout = nc.declare_dram_parameter("wout", [HPC * DK, D], bf16, isOutput=False)
    ones = nc.declare_dram_parameter("ones", [P, DK], bf16, isOutput=False)
    out = nc.declare_dram_parameter("out", [T, D], bf16, isOutput=True)

    with tile.TileContext(nc) as tc, ExitStack() as top:
        const = top.enter_context(tc.tile_pool(name="const", bufs=1))
        w_sb = const.tile([P, NDC, E], bf16)
        wout_sb = const.tile([P, 2, D], bf16)
        ident = const.tile([P, P], bf16)
        make_identity(nc, ident[:])
        ones_sb = const.tile([P, DK], f32)
        nc.gpsimd.memset(ones_sb[:], 1.0)
        x_sb = const.tile([P, NDC, T], bf16)  # x = queryT + posT, resident
        qkvT = const.tile([P, NEC, T], bf16)  # feature-major qkv projection
        # V token-major with a ones column per head: [t, (h, dk+1)]
        V_sb = const.tile([P, NTB, HPC, DK + 1], bf16)
        oT = const.tile([P, 2, T], bf16)  # normalized per-head-pair outputs
        r_pool = top.enter_context(tc.tile_pool(name="rr", bufs=3))
        rc_pool = top.enter_context(tc.tile_pool(name="rcp", bufs=3))
        dram_pool = top.enter_context(tc.tile_pool(name="sdp", bufs=4, space="DRAM"))
        osb_pool = top.enter_context(tc.tile_pool(name="osb", bufs=3))

        # reps>1 wraps the body in an on-device loop (timing builds only)
        rep_ctx = tc.For_i(0, reps, 1) if reps > 1 else nullcontext()
        top.enter_context(rep_ctx)

        # DMA priority order: x for query-half 0 first (sync+scalar queues in
        # parallel), weights next, second-half x behind, wout/ones last.
        xq3 = xqT.rearrange("(c p) t -> p c t", p=P)
        pos3 = posT.rearrange("(c p) t -> p c t", p=P)
        ld_pool = top.enter_context(tc.tile_pool(name="ldtmp", bufs=4))
        pts = [
            ld_pool.tile([P, NDC, 512], bf16, tag="ldp", name=f"pt{tg}")
            for tg in range(NTG)
        ]
        nc.sync.dma_start(x_sb[:, :, 0:512], xq3[:, :, 0:512])
        nc.scalar.dma_start(pts[0][:], pos3[:, :, 0:512])
        nc.scalar.dma_start(pts[1][:], pos3[:, :, 512:1024])
        nc.sync.dma_start(x_sb[:, :, 512:1024], xq3[:, :, 512:1024])
        nc.sync.dma_start(w_sb[:], wqkv.rearrange("(c p) e -> p c e", p=P))
        nc.scalar.dma_start(pts[2][:], pos3[:, :, 1024:1536])
        nc.sync.dma_start(x_sb[:, :, 1024:1536], xq3[:, :, 1024:1536])
        nc.scalar.dma_start(pts[3][:], pos3[:, :, 1536:2048])
        nc.sync.dma_start(x_sb[:, :, 1536:2048], xq3[:, :, 1536:2048])
        nc.scalar.dma_start(wout_sb[:], wout.rearrange("(c p) n -> p c n", p=P))
        for h in range(HPC):
            nc.scalar.dma_start(V_sb[:, :, h, DK : DK + 1], ones[:, 0:NTB])
        for tg in range(NTG):
            c0 = tg * 512
            nc.vector.tensor_add(
                x_sb[:, :, c0 : c0 + 512], x_sb[:, :, c0 : c0 + 512], pts[tg][:]
            )

        # ---- attention + deadline-scheduled PE filler ----
        with (
            tc.tile_pool(name="ptl", bufs=18) as pt_pool,
            tc.tile_pool(name="psqk", bufs=2, space="PSUM") as psum_qk,
            tc.tile_pool(name="psav", bufs=1, space="PSUM") as psum_av,
            tc.tile_pool(name="psfl", bufs=1, space="PSUM") as psum_fl,
            tc.tile_pool(name="pstr", bufs=1, space="PSUM") as psum_tr,
        ):
            # pre-warm the PE's HAM clock gate while the x DMAs stream in
            # (few enough that the serialized WAR chain ends before the first
            # projection's inputs land - 140 of these jammed the PE queue
            # for 45us in v5's first cut)
            for i in range(36):
                pw = psum_fl.tile([P, 512], f32, name="pwarm", tag="fil")
                nc.tensor.matmul(pw[:, 0:P], ident[:], ident[:], start=True, stop=True)

            vstate = {}

            def fill_proj(ec, tg, quarter):
                # one quarter (2 contraction steps) of projection (ec, tg)
                c0 = tg * 512
                if quarter == 0:
                    vstate["ps"] = psum_fl.tile([P, 512], f32, name="pfil", tag="fil")
                ps = vstate["ps"]
                for dc in range(quarter * 2, quarter * 2 + 2):
                    nc.tensor.matmul(
                        ps[:],
                        w_sb[:, dc, ec * P : (ec + 1) * P],
                        x_sb[:, dc, c0 : c0 + 512],
                        start=(dc == 0),
                        stop=(dc == NDC - 1),
                    )
                if quarter == 3:
                    nc.vector.tensor_copy(qkvT[:, ec, c0 : c0 + 512], ps[:])

            def fill_vtrans(evc, tb):
                pst = psum_tr.tile([P, P], bf16, name="ptr", tag="tr")
                nc.tensor.transpose(
                    pst[:], qkvT[:, 4 + evc, tb * P : (tb + 1) * P], ident[:]
                )
                nc.vector.tensor_copy(
                    V_sb[:, tb, 2 * evc : 2 * evc + 2, 0:DK],
                    pst.rearrange("p (h d) -> p h d", h=2),
                )

            ostate = {}

            def fill_oproj(tb, nh):
                # query-half-0 output projection, hidden under the exp stream
                tag = "fil" if nh == 0 else "tr"
                pool = psum_fl if nh == 0 else psum_tr
                po = pool.tile([P, 512], f32, name="pop", tag=tag)
                for pair in range(2):
                    nc.tensor.matmul(
                        po[:],
                        oT[:, pair, tb * P : (tb + 1) * P],
                        wout_sb[:, pair, nh * 512 : (nh + 1) * 512],
                        start=(pair == 0),
                        stop=(pair == 1),
                    )
                if nh == 0:
                    ostate[tb] = osb_pool.tile([P, D], bf16, name="ob", tag="ob")
                ob = ostate[tb]
                nc.vector.tensor_copy(ob[:, nh * 512 : (nh + 1) * 512], po[:])
                if nh == 1:
                    nc.sync.dma_start(out[tb * P : (tb + 1) * P, :], ob[:])

            def fill_dummy():
                ps = psum_fl.tile([P, 512], f32, name="pdum", tag="fil")
                nc.tensor.matmul(
                    ps[:], w_sb[:, 0, 0:P], x_sb[:, 0, 0:512],
                    start=True, stop=True,
                )

            def projq(ec, tg):
                return [("proj", ec, tg, q) for q in range(4)]

            # Filler items in deadline order; per-slot budgets below drain
            # them just in time (trans0 before AV(h0)@slot1, K columns before
            # QK(h2)@slot2, trans1 before AV(h2)@slot3, half-1 Q before
            # QK@slot4, then the half-0 output projection under slots 5-7).
            filler = projq(2, 2) + projq(2, 3)
            for tg in range(NTG):
                filler += projq(4, tg)
            filler += [("trans", 0, tb, 0) for tb in range(NTB)]
            filler += projq(1, 0) + projq(1, 1) + projq(3, 0) + projq(3, 1)
            filler += projq(3, 2) + projq(3, 3)
            for tg in range(NTG):
                filler += projq(5, tg)
            filler += [("trans", 1, tb, 0) for tb in range(NTB)]
            filler += projq(0, 2) + projq(0, 3) + projq(1, 2) + projq(1, 3)
            filler += [("oproj", tb, nh, 0) for tb in range(NTB // 2) for nh in range(2)]
            budgets = [32, 28, 24, 20, 8, 6, 6, 4]
            fill_i = 0

            def fill(si, kb):
                nonlocal fill_i
                b = budgets[si]
                n = (b * (kb + 1)) // NTB - (b * kb) // NTB
                emitted = 0
                while emitted < n and fill_i < len(filler):
                    item = filler[fill_i]
                    fill_i += 1
                    emitted += 1
                    if item[0] == "proj":
                        fill_proj(item[1], item[2], item[3])
                    elif item[0] == "trans":
                        fill_vtrans(item[1], item[2])
                    else:
                        fill_oproj(item[1], item[2])
                if emitted == 0:
                    fill_dummy()

            ptiles = {}

            def emit_qk(h, kb, qh):
                ecq, row = h // 2, (h % 2) * DK
                q0 = qh * QH
                ptile = pt_pool.tile([P, QH], bf16, name="ptile", tag="pt")
                pqk = psum_qk.tile([P, 1024], f32, name="pqk", tag="pqk")
                for qq in range(2):
                    nc.tensor.matmul(
                        pqk[:, qq * 512 : (qq + 1) * 512],
                        qkvT[row : row + DK, 2 + ecq, kb * P : (kb + 1) * P],
                        qkvT[row : row + DK, ecq, q0 + qq * 512 : q0 + (qq + 1) * 512],
                        start=True,
                        stop=True,
                    )
                nc.scalar.activation(ptile[:], pqk[:], EXP, scale=SCALE)
                ptiles[(h, kb, qh)] = ptile

            def emit_av(h, kb, qh, poT):
                ptile = ptiles.pop((h, kb, qh))
                for qq in range(2):
                    nc.tensor.matmul(
                        poT[:, qq * 512 : (qq + 1) * 512],
                        V_sb[:, kb, h, :],
                        ptile[:, qq * 512 : (qq + 1) * 512],
                        start=(kb == 0),
                        stop=(kb == NTB - 1),
                    )

            def evac_av(poT):
                o_us = r_pool.tile([DK + 1, QH], f32, tag="ous")
                nc.vector.tensor_copy(o_us[:], poT[:])
                return o_us

            def norm_dma(h, qh, o_us):
                ecq, row = h // 2, (h % 2) * DK
                q0 = qh * QH
                s_dram = dram_pool.tile([1, QH], f32, name="sdram", tag="sd")
                nc.sync.dma_start(s_dram[:], o_us[DK : DK + 1, :])
                rs = rc_pool.tile([DK, QH // DK], f32, tag="rs")
                nc.sync.dma_start(rs[:], s_dram.rearrange("o (p c) -> (o p) c", p=DK))
                nc.vector.reciprocal_approx_fast(rs[:], rs[:])
                s2_dram = dram_pool.tile([1, QH], f32, name="s2dram", tag="sd2")
                nc.sync.dma_start(s2_dram.rearrange("o (p c) -> (o p) c", p=DK), rs[:])
                rbc = r_pool.tile([DK, QH], f32, tag="rbc")
                nc.sync.dma_start(rbc[:], s2_dram[:].partition_broadcast(DK))
                nc.vector.tensor_mul(
                    oT[row : row + DK, ecq, q0 : q0 + QH], o_us[0:DK, :], rbc[:]
                )

            # the projections the first exps need, directly before the slots
            for ec, tg in ((0, 0), (2, 0), (0, 1), (2, 1)):
                for q in range(4):
                    fill_proj(ec, tg, q)

            slots = [(qh, h) for qh in range(2) for h in range(HPC)]
            prev = None  # (h, qh, poT) one slot behind
            for si, (qh, h) in enumerate(slots):
                for kb in range(NTB):
                    emit_qk(h, kb, qh)
                    fill(si, kb)
                    if prev is not None:
                        emit_av(prev[0], kb, prev[1], prev[2])
                if prev is not None:
                    norm_dma(prev[0], prev[1], evac_av(prev[2]))
                poT = psum_av.tile([DK + 1, QH], f32, name="poT", tag="po")
                prev = (h, qh, poT)
            for kb in range(NTB):
                emit_av(prev[0], kb, prev[1], prev[2])
            o_us_last = evac_av(prev[2])

        # ---- last head's normalize (PE broadcast, no DMA) + half-1 outproj ----
        with (
            tc.tile_pool(name="pso", bufs=4, space="PSUM") as psum_o,
            tc.tile_pool(name="psbc", bufs=1, space="PSUM") as psum_bc,
        ):
            pden = psum_bc.tile([DK, QH], f32, name="pden", tag="bc")
            nc.tensor.matmul(
                pden[:, 0:512], ones_sb[DK : DK + 1, :], o_us_last[DK : DK + 1, 0:512],
                start=True, stop=True,
            )
            nc.tensor.matmul(
                pden[:, 512:QH], ones_sb[DK : DK + 1, :], o_us_last[DK : DK + 1, 512:QH],
                start=True, stop=True,
            )
            rbc_sb = r_pool.tile([DK, QH], f32, tag="rbc")
            nc.vector.reciprocal_approx_fast(rbc_sb[:], pden[:])
            nc.vector.tensor_mul(
                oT[DK : 2 * DK, 1, QH:T], o_us_last[0:DK, :], rbc_sb[:]
            )

            for tb in range(NTB // 2, NTB):
                ob = osb_pool.tile([P, D], bf16, name="ob2", tag="ob")
                for nh in range(2):
                    po = psum_o.tile([P, 512], f32, name="po", tag="po")
                    for pair in range(2):
                        nc.tensor.matmul(
                            po[:],
                            oT[:, pair, tb * P : (tb + 1) * P],
                            wout_sb[:, pair, nh * 512 : (nh + 1) * 512],
                            start=(pair == 0),
                            stop=(pair == 1),
                        )
                    if nh == 0:
                        nc.vector.tensor_copy(ob[:, 0:512], po[:])
                    else:
                        nc.scalar.copy(ob[:, 512:1024], po[:])
                nc.sync.dma_start(out[tb * P : (tb + 1) * P, :], ob[:])

    nc.compile()
    return nc


def get_program():
    if "nc" not in _CACHE:
        _CACHE["nc"] = _build_program()
    return _CACHE["nc"]


def make_in_maps(query, pos_emb, w_qkv, w_out):
    import ml_dtypes

    bf16 = ml_dtypes.bfloat16
    query = np.asarray(query, dtype=np.float32)
    pos_emb = np.asarray(pos_emb, dtype=np.float32)
    w_qkv = np.asarray(w_qkv, dtype=np.float32)
    w_out = np.asarray(w_out, dtype=np.float32)
    posT = np.ascontiguousarray(pos_emb.T).astype(bf16)
    xqTs = [np.ascontiguousarray(query[b].T).astype(bf16) for b in range(B)]
    in_maps = []
    for c in range(NCORES):
        b, hb = c // (NCORES // B), (c % (NCORES // B)) * HPC
        heads = range(hb, hb + HPC)
        # w_qkv column e for head h, kind j (q/k/v), dim d: e = h*3*DK + j*DK + d
        wq_c = np.concatenate(
            [w_qkv[:, h * 3 * DK + j * DK : h * 3 * DK + (j + 1) * DK] for j in range(3) for h in heads],
            axis=1,
        )
        wout_c = np.concatenate([w_out[h * DK : (h + 1) * DK, :] for h in heads], axis=0)
        in_maps.append(
            {
                "xqT": xqTs[b],
                "posT": posT,
                "wqkv": np.ascontiguousarray(wq_c).astype(bf16),
                "wout": np.ascontiguousarray(wout_c).astype(bf16),
                "ones": np.ones((P, DK), dtype=bf16),
            }
        )
    return in_maps


def gather_output(results):
    out = np.zeros((B, S, D), dtype=np.float32)
    for c in range(NCORES):
        out[c // (NCORES // B)] += np.asarray(results[c]["out"], dtype=np.float32)
    return out


def kernel(query, pos_emb, w_qkv, w_out):
    from concourse.bass_utils import run_bass_kernel_spmd

    nc = get_program()
    in_maps = make_in_maps(query, pos_emb, w_qkv, w_out)
    res = run_bass_kernel_spmd(nc, in_maps, list(range(NCORES)))
    return gather_output(res.results)


# revision 15
# speedup vs baseline: 1.0390x; 1.0390x over previous
"""Multi-headed self-attention (B=2, S=2048, D=1024, H=16) on 8 TRN2 cores.

Sharding: hybrid batch x head tensor-parallel. Core c handles batch c//4 and
heads (c%4)*4 .. (c%4)*4+3. Each core computes x = query[b] + pos_emb, the QKV
projection for its 4 heads, attention, and a partial output projection
(o_heads @ w_out_rows). Host sums the 4 partials per batch.

v5 design notes (fp32r baseline ~410us, v2 ~304, v3 ~279, v4 ~260):
- The kernel is ScalarE-bound: 4 heads x 2048^2 exps = 16.8M elements at
  1 elem/lane/cycle @ 1.2 GHz is a ~142us floor (N=1024 chunks). Everything
  else hides under the exp stream, which starts as early as the input DMAs
  allow (~4.5MB of critical bytes at ~390 GB/s) and runs wall-to-wall.
- Attention is a flat pipeline over 8 slots (2 query-halves x 4 heads); AV
  of slot i runs under QK+exp of slot i+1; exp'ed scores (ptiles) buffer in
  SBUF bf16 to decouple the chains. Only the projections the first exps
  need (Q half-0 and the first K columns) precede the stream; every other
  projection, the V transposes, and the ENTIRE query-half-0 output
  projection are deadline-scheduled PE filler inside the stream, then
  dummy matmuls keep the PE dense so the HAM clock gate stays at K=8/8
  (the fp32r baseline sat throttled at 1.2 GHz for its whole attention
  phase; identity-matmul dummies also pre-warm during the DMA head).
- PSUM: score tiles 2x[128,1024] (4 banks) + AV accumulator [65,1024]
  (2 banks) + two filler slots (2 banks) = 8.
- Softmax denominators ride the AV matmul as a ones-column (row DK of the
  accumulator). Off-critical-path heads: reciprocal on a [64,16] reshape of
  the denominator row + DMA partition-broadcast. The last head (critical
  path into the remaining output projection) broadcasts via a rank-1 PE
  matmul + reciprocal_approx_fast instead - no DMA round trip.
- Softmax skips the max-subtraction: scores * dk^-0.5 are bounded (~|12|)
  for these inputs, well within exp range.
"""

import os
import sys

import numpy as np

if "/opt/trn_rl_repo" not in sys.path:
    sys.path.insert(0, "/opt/trn_rl_repo")

B, S, D, H = 2, 2048, 1024, 16
DK = 64
P = 128
NCORES = 8
HPC = H // (NCORES // B)  # heads per core = 4
T = S  # tokens per core (one batch)
E = HPC * 3 * DK  # 768 qkv output columns per core
NDC = D // P  # 8 contraction chunks
NEC = E // P  # 6 projection output chunks
NTB = T // P  # 16 token blocks
NTG = T // 512  # 4 token groups of 512
QH = T // 2  # query half
SCALE = DK**-0.5

_CACHE = {}


def _build_program(reps=1):
    from contextlib import ExitStack, nullcontext

    import concourse.bass as bass
    import concourse.tile as tile
    from concourse import bacc
    from concourse import mybir
    from concourse.masks import make_identity

    f32 = mybir.dt.float32
    bf16 = mybir.dt.bfloat16
    EXP = mybir.ActivationFunctionType.Exp

    nc = bacc.Bacc()
    xqT = nc.declare_dram_parameter("xqT", [D, T], bf16, isOutput=False)
    posT = nc.declare_dram_parameter("posT", [D, T], bf16, isOutput=False)
    wqkv = nc.declare_dram_parameter("wqkv", [D, E], bf16, isOutput=False)
    wout = nc.declare_dram_parameter("wout", [HPC * DK, D], bf16, isOutput=False)
    ones = nc.declare_dram_parameter("ones", [P, DK], bf16, isOutput=False)
    out = nc.declare_dram_parameter("out", [T, D], bf16, isOutput=True)

    with tile.TileContext(nc) as tc, ExitStack() as top:
        const = top.enter_context(tc.tile_pool(name="const", bufs=1))
        w_sb = const.tile([P, NDC, E], bf16)
        wout_sb = const.tile([P, 2, D], bf16)
        ident = const.tile([P, P], bf16)
        make_identity(nc, ident[:])
        ones_sb = const.tile([P, DK], f32)
        nc.gpsimd.memset(ones_sb[:], 1.0)
        x_sb = const.tile([P, NDC, T], bf16)  # x = queryT + posT, resident
        qkvT = const.tile([P, NEC, T], bf16)  # feature-major qkv projection
        # V token-major with a ones column per head: [t, (h, dk+1)]
        V_sb = const.tile([P, NTB, HPC, DK + 1], bf16)
        oT = const.tile([P, 2, T], bf16)  # normalized per-head-pair outputs
        r_pool = top.enter_context(tc.tile_pool(name="rr", bufs=3))
        rc_pool = top.enter_context(tc.tile_pool(name="rcp", bufs=3))
        dram_pool = top.enter_context(tc.tile_pool(name="sdp", bufs=4, space="DRAM"))
        osb_pool = top.enter_context(tc.tile_pool(name="osb", bufs=3))

        # reps>1 wraps the body in an on-device loop (timing builds only)
        rep_ctx = tc.For_i(0, reps, 1) if reps > 1 else nullcontext()
        top.enter_context(rep_ctx)

        # DMA priority order: x for query-half 0 first (sync+scalar queues in
        # parallel), weights next, second-half x behind, wout/ones last.
        xq3 = xqT.rearrange("(c p) t -> p c t", p=P)
        pos3 = posT.rearrange("(c p) t -> p c t", p=P)
        ld_pool = top.enter_context(tc.tile_pool(name="ldtmp", bufs=4))
        pts = [
            ld_pool.tile([P, NDC, 512], bf16, tag="ldp", name=f"pt{tg}")
            for tg in range(NTG)
        ]
        nc.sync.dma_start(x_sb[:, :, 0:512], xq3[:, :, 0:512])
        nc.scalar.dma_start(pts[0][:], pos3[:, :, 0:512])
        nc.scalar.dma_start(pts[1][:], pos3[:, :, 512:1024])
        nc.sync.dma_start(x_sb[:, :, 512:1024], xq3[:, :, 512:1024])
        nc.sync.dma_start(w_sb[:], wqkv.rearrange("(c p) e -> p c e", p=P))
        nc.scalar.dma_start(pts[2][:], pos3[:, :, 1024:1536])
        nc.sync.dma_start(x_sb[:, :, 1024:1536], xq3[:, :, 1024:1536])
        nc.scalar.dma_start(pts[3][:], pos3[:, :, 1536:2048])
        nc.sync.dma_start(x_sb[:, :, 1536:2048], xq3[:, :, 1536:2048])
        nc.scalar.dma_start(wout_sb[:], wout.rearrange("(c p) n -> p c n", p=P))
        for h in range(HPC):
            nc.scalar.dma_start(V_sb[:, :, h, DK : DK + 1], ones[:, 0:NTB])
        def add_x(tg):
            c0 = tg * 512
            nc.vector.tensor_add(
                x_sb[:, :, c0 : c0 + 512], x_sb[:, :, c0 : c0 + 512], pts[tg][:]
            )

        # tg0/tg1 adds only - tg2/tg3 are emitted after the first projections
        # so their x-DMA waits don't block the in-order DVE queue ahead of the
        # projection evacuation copies the first exps depend on
        add_x(0)
        add_x(1)

        # ---- attention + deadline-scheduled PE filler ----
        with (
            tc.tile_pool(name="ptl", bufs=18) as pt_pool,
            tc.tile_pool(name="psqk", bufs=2, space="PSUM") as psum_qk,
            tc.tile_pool(name="psav", bufs=1, space="PSUM") as psum_av,
            tc.tile_pool(name="psfl", bufs=1, space="PSUM") as psum_fl,
            tc.tile_pool(name="pstr", bufs=1, space="PSUM") as psum_tr,
        ):
            # pre-warm the PE's HAM clock gate while the x DMAs stream in
            # (few enough that the serialized WAR chain ends before the first
            # projection's inputs land - 140 of these jammed the PE queue
            # for 45us in v5's first cut)
            for i in range(36):
                pw = psum_fl.tile([P, 512], f32, name="pwarm", tag="fil")
                nc.tensor.matmul(pw[:, 0:P], ident[:], ident[:], start=True, stop=True)

            vstate = {}

            def fill_proj(ec, tg, quarter):
                # one quarter (2 contraction steps) of projection (ec, tg)
                c0 = tg * 512
                if quarter == 0:
                    vstate["ps"] = psum_fl.tile([P, 512], f32, name="pfil", tag="fil")
                ps = vstate["ps"]
                for dc in range(quarter * 2, quarter * 2 + 2):
                    nc.tensor.matmul(
                        ps[:],
                        w_sb[:, dc, ec * P : (ec + 1) * P],
                        x_sb[:, dc, c0 : c0 + 512],
                        start=(dc == 0),
                        stop=(dc == NDC - 1),
                    )
                if quarter == 3:
                    nc.vector.tensor_copy(qkvT[:, ec, c0 : c0 + 512], ps[:])

            def fill_vtrans(evc, tb):
                pst = psum_tr.tile([P, P], bf16, name="ptr", tag="tr")
                nc.tensor.transpose(
                    pst[:], qkvT[:, 4 + evc, tb * P : (tb + 1) * P], ident[:]
                )
                nc.vector.tensor_copy(
                    V_sb[:, tb, 2 * evc : 2 * evc + 2, 0:DK],
                    pst.rearrange("p (h d) -> p h d", h=2),
                )

            ostate = {}

            def fill_oproj(tb, nh):
                # query-half-0 output projection, hidden under the exp stream
                tag = "fil" if nh == 0 else "tr"
                pool = psum_fl if nh == 0 else psum_tr
                po = pool.tile([P, 512], f32, name="pop", tag=tag)
                for pair in range(2):
                    nc.tensor.matmul(
                        po[:],
                        oT[:, pair, tb * P : (tb + 1) * P],
                        wout_sb[:, pair, nh * 512 : (nh + 1) * 512],
                        start=(pair == 0),
                        stop=(pair == 1),
                    )
                if nh == 0:
                    ostate[tb] = osb_pool.tile([P, D], bf16, name="ob", tag="ob")
                ob = ostate[tb]
                nc.vector.tensor_copy(ob[:, nh * 512 : (nh + 1) * 512], po[:])
                if nh == 1:
                    nc.sync.dma_start(out[tb * P : (tb + 1) * P, :], ob[:])

            def fill_dummy():
                ps = psum_fl.tile([P, 512], f32, name="pdum", tag="fil")
                nc.tensor.matmul(
                    ps[:], w_sb[:, 0, 0:P], x_sb[:, 0, 0:512],
                    start=True, stop=True,
                )

            def projq(ec, tg):
                return [("proj", ec, tg, q) for q in range(4)]

            # Filler items in deadline order; per-slot budgets below drain
            # them just in time (trans0 before AV(h0)@slot1, K columns before
            # QK(h2)@slot2, trans1 before AV(h2)@slot3, half-1 Q before
            # QK@slot4, then the half-0 output projection under slots 5-7).
            filler = projq(2, 2) + projq(2, 3)
            for tg in range(NTG):
                filler += projq(4, tg)
            filler += [("trans", 0, tb, 0) for tb in range(NTB)]
            filler += projq(1, 0) + projq(1, 1) + projq(3, 0) + projq(3, 1)
            filler += projq(3, 2) + projq(3, 3)
            for tg in range(NTG):
                filler += projq(5, tg)
            filler += [("trans", 1, tb, 0) for tb in range(NTB)]
            filler += projq(0, 2) + projq(0, 3) + projq(1, 2) + projq(1, 3)
            filler += [("oproj", tb, nh, 0) for tb in range(NTB // 2) for nh in range(2)]
            budgets = [32, 28, 24, 20, 8, 6, 6, 4]
            fill_i = 0

            def fill(si, kb):
                nonlocal fill_i
                b = budgets[si]
                n = (b * (kb + 1)) // NTB - (b * kb) // NTB
                emitted = 0
                while emitted < n and fill_i < len(filler):
                    item = filler[fill_i]
                    fill_i += 1
                    emitted += 1
                    if item[0] == "proj":
                        fill_proj(item[1], item[2], item[3])
                    elif item[0] == "trans":
                        fill_vtrans(item[1], item[2])
                    else:
                        fill_oproj(item[1], item[2])
                if emitted == 0:
                    fill_dummy()

            ptiles = {}

            def emit_qk(h, kb, qh):
                ecq, row = h // 2, (h % 2) * DK
                q0 = qh * QH
                ptile = pt_pool.tile([P, QH], bf16, name="ptile", tag="pt")
                pqk = psum_qk.tile([P, 1024], f32, name="pqk", tag="pqk")
                for qq in range(2):
                    nc.tensor.matmul(
                        pqk[:, qq * 512 : (qq + 1) * 512],
                        qkvT[row : row + DK, 2 + ecq, kb * P : (kb + 1) * P],
                        qkvT[row : row + DK, ecq, q0 + qq * 512 : q0 + (qq + 1) * 512],
                        start=True,
                        stop=True,
                    )
                nc.scalar.activation(ptile[:], pqk[:], EXP, scale=SCALE)
                ptiles[(h, kb, qh)] = ptile

            def emit_av(h, kb, qh, poT):
                ptile = ptiles.pop((h, kb, qh))
                for qq in range(2):
                    nc.tensor.matmul(
                        poT[:, qq * 512 : (qq + 1) * 512],
                        V_sb[:, kb, h, :],
                        ptile[:, qq * 512 : (qq + 1) * 512],
                        start=(kb == 0),
                        stop=(kb == NTB - 1),
                    )

            def evac_av(poT):
                o_us = r_pool.tile([DK + 1, QH], f32, tag="ous")
                nc.vector.tensor_copy(o_us[:], poT[:])
                return o_us

            def norm_dma(h, qh, o_us):
                ecq, row = h // 2, (h % 2) * DK
                q0 = qh * QH
                s_dram = dram_pool.tile([1, QH], f32, name="sdram", tag="sd")
                nc.sync.dma_start(s_dram[:], o_us[DK : DK + 1, :])
                rs = rc_pool.tile([DK, QH // DK], f32, tag="rs")
                nc.sync.dma_start(rs[:], s_dram.rearrange("o (p c) -> (o p) c", p=DK))
                nc.vector.reciprocal_approx_fast(rs[:], rs[:])
                s2_dram = dram_pool.tile([1, QH], f32, name="s2dram", tag="sd2")
                nc.sync.dma_start(s2_dram.rearrange("o (p c) -> (o p) c", p=DK), rs[:])
                rbc = r_pool.tile([DK, QH], f32, tag="rbc")
                nc.sync.dma_start(rbc[:], s2_dram[:].partition_broadcast(DK))
                nc.vector.tensor_mul(
                    oT[row : row + DK, ecq, q0 : q0 + QH], o_us[0:DK, :], rbc[:]
                )

            # the projections the first exps need, directly before the slots
            for ec, tg in ((0, 0), (2, 0), (0, 1), (2, 1)):
                for q in range(4):
                    fill_proj(ec, tg, q)
            add_x(2)
            add_x(3)

            slots = [(qh, h) for qh in range(2) for h in range(HPC)]
            prev = None  # (h, qh, poT) one slot behind
            for si, (qh, h) in enumerate(slots):
                for kb in range(NTB):
                    emit_qk(h, kb, qh)
                    fill(si, kb)
                    if prev is not None:
                        emit_av(prev[0], kb, prev[1], prev[2])
                if prev is not None:
                    norm_dma(prev[0], prev[1], evac_av(prev[2]))
                poT = psum_av.tile([DK + 1, QH], f32, name="poT", tag="po")
                prev = (h, qh, poT)
            for kb in range(NTB):
                emit_av(prev[0], kb, prev[1], prev[2])
            o_us_last = evac_av(prev[2])

        # ---- last head's normalize (PE broadcast, no DMA) + half-1 outproj ----
        with (
            tc.tile_pool(name="pso", bufs=4, space="PSUM") as psum_o,
            tc.tile_pool(name="psbc", bufs=1, space="PSUM") as psum_bc,
        ):
            pden = psum_bc.tile([DK, QH], f32, name="pden", tag="bc")
            nc.tensor.matmul(
                pden[:, 0:512], ones_sb[DK : DK + 1, :], o_us_last[DK : DK + 1, 0:512],
                start=True, stop=True,
            )
            nc.tensor.matmul(
                pden[:, 512:QH], ones_sb[DK : DK + 1, :], o_us_last[DK : DK + 1, 512:QH],
                start=True, stop=True,
            )
            rbc_sb = r_pool.tile([DK, QH], f32, tag="rbc")
            nc.vector.reciprocal_approx_fast(rbc_sb[:], pden[:])
            nc.vector.tensor_mul(
                oT[DK : 2 * DK, 1, QH:T], o_us_last[0:DK, :], rbc_sb[:]
            )

            for tb in range(NTB // 2, NTB):
                ob = osb_pool.tile([P, D], bf16, name="ob2", tag="ob")
                for nh in range(2):
                    po = psum_o.tile([P, 512], f32, name="po", tag="po")
                    for pair in range(2):
                        nc.tensor.matmul(
                            po[:],
                            oT[:, pair, tb * P : (tb + 1) * P],
                            wout_sb[:, pair, nh * 512 : (nh + 1) * 512],
                            start=(pair == 0),
                            stop=(pair == 1),
                        )
                    if nh == 0:
                        nc.vector.tensor_copy(ob[:, 0:512], po[:])
                    else:
                        nc.scalar.copy(ob[:, 512:1024], po[:])
                nc.sync.dma_start(out[tb * P : (tb + 1) * P, :], ob[:])

    nc.compile()
    return nc


def get_program():
    if "nc" not in _CACHE:
        _CACHE["nc"] = _build_program()
    return _CACHE["nc"]


def make_in_maps(query, pos_emb, w_qkv, w_out):
    import ml_dtypes

    bf16 = ml_dtypes.bfloat16
    query = np.asarray(query, dtype=np.float32)
    pos_emb = np.asarray(pos_emb, dtype=np.float32)
    w_qkv = np.asarray(w_qkv, dtype=np.float32)
    w_out = np.asarray(w_out, dtype=np.float32)
    posT = np.ascontiguousarray(pos_emb.T).astype(bf16)
    xqTs = [np.ascontiguousarray(query[b].T).astype(bf16) for b in range(B)]
    in_maps = []
    for c in range(NCORES):
        b, hb = c // (NCORES // B), (c % (NCORES // B)) * HPC
        heads = range(hb, hb + HPC)
        # w_qkv column e for head h, kind j (q/k/v), dim d: e = h*3*DK + j*DK + d
        wq_c = np.concatenate(
            [w_qkv[:, h * 3 * DK + j * DK : h * 3 * DK + (j + 1) * DK] for j in range(3) for h in heads],
            axis=1,
        )
        wout_c = np.concatenate([w_out[h * DK : (h + 1) * DK, :] for h in heads], axis=0)
        in_maps.append(
            {
                "xqT": xqTs[b],
                "posT": posT,
                "wqkv": np.ascontiguousarray(wq_c).astype(bf16),
                "wout": np.ascontiguousarray(wout_c).astype(bf16),
                "ones": np.ones((P, DK), dtype=bf16),
            }
        )
    return in_maps


def gather_output(results):
    out = np.zeros((B, S, D), dtype=np.float32)
    for c in range(NCORES):
        out[c // (NCORES // B)] += np.asarray(results[c]["out"], dtype=np.float32)
    return out


def kernel(query, pos_emb, w_qkv, w_out):
    from concourse.bass_utils import run_bass_kernel_spmd

    nc = get_program()
    in_maps = make_in_maps(query, pos_emb, w_qkv, w_out)
    res = run_bass_kernel_spmd(nc, in_maps, list(range(NCORES)))
    return gather_output(res.results)
